# revision 23
# baseline (speedup 1.0000x reference)
"""Causal self-attention (B=2, T=2048, C=1024, H=16) on 8 trn2 NeuronCores.

Sharding: core c -> batch b=c//4 (data parallel) x head-group g=c%4
(tensor parallel, 4 heads each). Each core computes qkv projections for
its 4 heads, causal flash-style attention, and a partial output
projection (its heads' rows of W_proj); the host sums the 4 partials
per batch and adds b_proj.

Device layout avoids all on-chip transposes:
  - q,k are produced transposed ([head_dim*heads, T], dims on partitions)
    straight out of the qkv matmul (W as stationary, xT as moving).
  - v is produced in natural [T, head_dim] layout (xT as stationary),
    augmented with a ones column so P@v_aug also yields the softmax
    denominator.
  - scores are computed as S^T = k @ q^T in [s, t] layout, exp'ed with
    no max subtraction (scores are O(1); fully-masked blocks are
    skipped, diagonal blocks multiplied by a 0/1 triangular mask, which
    matches exp(-10000+...) == 0.0 in fp32 exactly).
  - y^T = v_aug^T @ P accumulates in PSUM; the denominator row is
    reciprocal'ed and broadcast across partitions via a tiny selector
    matmul, then multiplied in.
  - the output projection consumes y^T directly as the moving operand
    (W_proj slice stationary), producing the partial output transposed.

All matmul operands are float32r (1 cycle/row at N>=512 vs 4 for fp32;
measured dot-product rel-err ~1.5e-4 at K=128).
"""

import numpy as np

import concourse.bass as bass
import concourse.tile as tile
from concourse import bacc, mybir
from concourse.bass_utils import run_bass_kernel_spmd

F32 = mybir.dt.float32
F32R = mybir.dt.float32r
EXP = mybir.ActivationFunctionType.Exp
IDENT = mybir.ActivationFunctionType.Identity

B, T, C, H = 2, 2048, 1024, 16
D = C // H                    # 64
N_CORES = 8
HG = 4                        # heads per core
TCH = 512                     # t-chunk (moving free dim)
NJ = T // TCH                 # 4 t-chunks
NS = T // 128                 # 16 s-tiles
KC = C // 128                 # 8 contraction chunks


def _emit(nc, tc, io):
    import contextlib
    ctx = contextlib.ExitStack()
    with ctx:
        const = ctx.enter_context(tc.tile_pool(name="const", bufs=1))
        xp = ctx.enter_context(tc.tile_pool(name="xp", bufs=8))
        qkp = ctx.enter_context(tc.tile_pool(name="qkp", bufs=1))
        vp = ctx.enter_context(tc.tile_pool(name="vp", bufs=1))
        pp = ctx.enter_context(tc.tile_pool(name="pp", bufs=4))
        yp = ctx.enter_context(tc.tile_pool(name="yp", bufs=1))
        op = ctx.enter_context(tc.tile_pool(name="op", bufs=3))
        ps = ctx.enter_context(tc.tile_pool(name="ps", bufs=6, space="PSUM"))

        # ---- constants / weights ----
        wqk_t = []
        wv_t = []
        for c in range(KC):
            w1 = const.tile([128, 512], F32R, name=f"wqk{c}")
            nc.sync.dma_start(w1[:], io["wqk"][128 * c:128 * (c + 1), :])
            wqk_t.append(w1)
            w2 = const.tile([128, 256], F32R, name=f"wv{c}")
            nc.sync.dma_start(w2[:], io["wv"][128 * c:128 * (c + 1), :])
            wv_t.append(w2)
        wp_t = []
        for p in range(2):
            w3 = const.tile([128, 1024], F32R, name=f"wp{p}")
            nc.sync.dma_start(w3[:], io["wp"][128 * p:128 * (p + 1), :])
            wp_t.append(w3)
        bqk_t = const.tile([128, 4], F32, name="bqk")
        nc.sync.dma_start(bqk_t[:], io["bqk"][:])
        bv_t = const.tile([128, 256], F32, name="bv")
        nc.sync.dma_start(bv_t[:], io["bv"][:])
        am_t = const.tile([128, NS], F32, name="am")
        nc.sync.dma_start(am_t[:], io["amask"][:])
        cm_t = const.tile([128, 2048], F32R, name="cm")
        nc.sync.dma_start(cm_t[:], io["cmask"][:])
        ones_t = const.tile([128, 64], F32R, name="ones")
        nc.sync.dma_start(ones_t[:], io["ones"][:])

        # ---- qkv outputs ----
        # qk_tiles m-chunks: 0: qT heads {0,1}; 1: qT heads {2,3};
        #                    2: kT heads {0,1}; 3: kT heads {2,3}
        qk_tiles = [qkp.tile([128, T], F32R, name=f"qk{m}") for m in range(4)]
        v_big = [vp.tile([128, 65 * NS], F32R, name=f"vb{h}") for h in range(HG)]
        for h in range(HG):
            # ones column at position 64 of every 65-wide block (softmax
            # denominator accumulator); memset can't write f32r, so copy
            # from the ones tile through a strided AP.
            onescol = v_big[h][:].rearrange("p (s c) -> p s c", c=65)[:, :, 64]
            nc.vector.tensor_copy(onescol, ones_t[:, 0:NS])

        # ---- qkv projection, in two t-halves to bound xT residency ----
        for half in range(2):
            xt = []
            for c in range(KC):
                x1 = xp.tile([128, 1024], F32R, name="xt", tag="xt")
                nc.sync.dma_start(
                    x1[:], io["xT"][128 * c:128 * (c + 1),
                                    1024 * half:1024 * (half + 1)])
                xt.append(x1)
            # qT / kT: weights stationary, xT moving -> transposed outputs
            for m in range(4):
                for tj in range(2):
                    j = 2 * half + tj
                    pq = ps.tile([128, 512], F32, name="pq", tag="ps")
                    for c in range(KC):
                        nc.tensor.matmul(
                            pq[:], wqk_t[c][:, 128 * m:128 * (m + 1)],
                            xt[c][:, 512 * tj:512 * (tj + 1)],
                            start=(c == 0), stop=(c == KC - 1))
                    nc.scalar.activation(
                        qk_tiles[m][:, TCH * j:TCH * (j + 1)], pq[:], IDENT,
                        bias=bqk_t[:, m:m + 1], scale=1.0)
            # v: xT stationary, Wv moving -> natural [s, d] layout
            for si in range(NS // 2):
                s = NS // 2 * half + si
                pv = ps.tile([128, 256], F32, name="pv", tag="ps")
                for c in range(KC):
                    nc.tensor.matmul(
                        pv[:], xt[c][:, 128 * si:128 * (si + 1)], wv_t[c][:],
                        start=(c == 0), stop=(c == KC - 1))
                for h in range(HG):
                    nc.vector.tensor_add(
                        v_big[h][:, 65 * s:65 * s + 64],
                        pv[:, 64 * h:64 * (h + 1)], bv_t[:, 64 * h:64 * (h + 1)])

        # ---- attention ----
        yT = [yp.tile([128, T], F32R, name=f"yT{p}") for p in range(2)]
        # denominator rows: head h -> tile h//2, partition 32*(h%2)
        # (engine APs may only start at partition 0/32/64; 96 is illegal)
        l_t = [yp.tile([64, T], F32, name=f"l{p}") for p in range(2)]
        rl_t = [yp.tile([64, T], F32R, name=f"rl{p}") for p in range(2)]

        for pr in range(2):
            qt, kt = qk_tiles[pr], qk_tiles[2 + pr]
            for hh in range(2):
                h = 2 * pr + hh
                rows = slice(64 * hh, 64 * (hh + 1))
                for j in range(NJ):
                    py = ps.tile([128, 512], F32, name="py", tag="ps")
                    ns = 4 * (j + 1)
                    for i in range(ns):
                        pscr = ps.tile([128, 512], F32, name="pscr", tag="ps")
                        nc.tensor.matmul(
                            pscr[:], kt[rows, 128 * i:128 * (i + 1)],
                            qt[rows, TCH * j:TCH * (j + 1)],
                            start=True, stop=True)
                        pt = pp.tile([128, 512], F32R, name="pt", tag="pt")
                        nc.scalar.activation(
                            pt[:], pscr[:], EXP,
                            bias=am_t[:, i:i + 1], scale=1.0 / np.sqrt(D))
                        r = i - 4 * j
                        if r >= 0:  # diagonal block: 0/1 triangular mask
                            nc.vector.tensor_mul(
                                pt[:], pt[:], cm_t[:, 512 * r:512 * (r + 1)])
                        nc.tensor.matmul(
                            py[0:65, :], v_big[h][:, 65 * i:65 * (i + 1)],
                            pt[:], start=(i == 0), stop=(i == ns - 1))
                    # drain y rows + denominator row, reciprocal, then
                    # broadcast the recip row across 64 partitions via a
                    # K=1 ones-row matmul and normalize in place.
                    cols = slice(TCH * j, TCH * (j + 1))
                    lr = 32 * hh
                    nc.vector.tensor_copy(yT[pr][rows, cols], py[0:64, :])
                    nc.vector.tensor_copy(l_t[pr][lr:lr + 1, cols],
                                          py[64:65, :])
                    nc.vector.reciprocal(rl_t[pr][lr:lr + 1, cols],
                                         l_t[pr][lr:lr + 1, cols])
                    pb = ps.tile([128, 512], F32, name="pb", tag="ps")
                    nc.tensor.matmul(
                        pb[0:64, :], ones_t[lr:lr + 1, :],
                        rl_t[pr][lr:lr + 1, cols],
                        start=True, stop=True)
                    nc.vector.tensor_mul(
                        yT[pr][rows, cols], yT[pr][rows, cols], pb[0:64, :])

        # ---- output projection (partial; host sums across head groups) ----
        for j in range(NJ):
            for m in range(8):
                po = ps.tile([128, 512], F32, name="po", tag="ps")
                for pr in range(2):
                    nc.tensor.matmul(
                        po[:], wp_t[pr][:, 128 * m:128 * (m + 1)],
                        yT[pr][:, TCH * j:TCH * (j + 1)],
                        start=(pr == 0), stop=(pr == 1))
                ot = op.tile([128, 512], F32, name="ot", tag="ot")
                nc.vector.tensor_copy(ot[:], po[:])
                nc.sync.dma_start(
                    io["out"][128 * m:128 * (m + 1), TCH * j:TCH * (j + 1)],
                    ot[:])


def _build():
    nc = bacc.Bacc("TRN2", target_bir_lowering=False, debug=False)
    io = {
        "xT": nc.dram_tensor("xT", [C, T], F32R, kind="ExternalInput").ap(),
        "wqk": nc.dram_tensor("wqk", [C, 512], F32R, kind="ExternalInput").ap(),
        "wv": nc.dram_tensor("wv", [C, 256], F32R, kind="ExternalInput").ap(),
        "wp": nc.dram_tensor("wp", [256, C], F32R, kind="ExternalInput").ap(),
        "bqk": nc.dram_tensor("bqk", [128, 4], F32, kind="ExternalInput").ap(),
        "bv": nc.dram_tensor("bv", [128, 256], F32, kind="ExternalInput").ap(),
        "amask": nc.dram_tensor("amask", [128, NS], F32, kind="ExternalInput").ap(),
        "cmask": nc.dram_tensor("cmask", [128, 2048], F32R, kind="ExternalInput").ap(),
        "ones": nc.dram_tensor("ones", [128, 64], F32R, kind="ExternalInput").ap(),
        "out": nc.dram_tensor("out", [C, T], F32, kind="ExternalOutput").ap(),
    }
    with nc.allow_low_precision("f32r matmul operand staging"):
        with tile.TileContext(nc) as tc:
            _emit(nc, tc, io)
    nc.compile()
    return nc


_NC_CACHE = {}


def _get_nc():
    if "nc" not in _NC_CACHE:
        _NC_CACHE["nc"] = _build()
    return _NC_CACHE["nc"]


def _host_inputs(x, attention_mask, W_attn, b_attn, W_proj):
    """Per-core input dicts implementing the batch x head-group sharding."""
    # causal 0/1 masks for the 4 diagonal-block offsets, as one [128, 2048]
    p = np.arange(128)[:, None]
    f = np.arange(512)[None, :]
    cm = np.concatenate(
        [(f >= 128 * r + p).astype(np.float32) for r in range(4)], axis=1)
    ones = np.ones((128, 64), np.float32)
    in_maps = []
    for c in range(N_CORES):
        b, g = divmod(c, HG)
        q0 = 256 * g
        wqk = np.ascontiguousarray(np.concatenate(
            [W_attn[:, q0:q0 + 256], W_attn[:, C + q0:C + q0 + 256]], axis=1))
        wv = np.ascontiguousarray(W_attn[:, 2 * C + q0:2 * C + q0 + 256])
        wp = np.ascontiguousarray(W_proj[q0:q0 + 256, :])
        bqk = np.stack(
            [b_attn[q0:q0 + 128], b_attn[q0 + 128:q0 + 256],
             b_attn[C + q0:C + q0 + 128], b_attn[C + q0 + 128:C + q0 + 256]],
            axis=1).astype(np.float32)
        bv = np.broadcast_to(
            b_attn[2 * C + q0:2 * C + q0 + 256], (128, 256)).astype(np.float32)
        am = np.ascontiguousarray(
            attention_mask[b, 0, 0].reshape(NS, 128).T.astype(np.float32))
        xT = np.ascontiguousarray(x[b].T)
        in_maps.append(dict(xT=xT, wqk=wqk, wv=wv, wp=wp, bqk=bqk, bv=bv,
                            amask=am, cmask=cm, ones=ones))
    return in_maps


def _assemble(results, b_proj):
    out = np.empty((B, T, C), np.float32)
    for b in range(B):
        acc = np.zeros((C, T), np.float64)
        for g in range(HG):
            acc += results[HG * b + g]["out"].astype(np.float64)
        out[b] = acc.T + b_proj[None, :]
    return out


def kernel(x, attention_mask, W_attn, b_attn, W_proj, b_proj):
    x = np.asarray(x, np.float32)
    attention_mask = np.asarray(attention_mask, np.float32)
    W_attn = np.asarray(W_attn, np.float32)
    b_attn = np.asarray(b_attn, np.float32)
    W_proj = np.asarray(W_proj, np.float32)
    b_proj = np.asarray(b_proj, np.float32)

    nc = _get_nc()
    in_maps = _host_inputs(x, attention_mask, W_attn, b_attn, W_proj)
    res = run_bass_kernel_spmd(nc, in_maps, list(range(N_CORES)))
    return _assemble(res.results, b_proj)
